# revision 14
# baseline (speedup 1.0000x reference)
"""Trainium2 Bass kernel for causal multi-head attention with RoPE.

Sharding: tensor-parallel over heads. 16 heads / 8 cores = 2 heads per core.
Each core computes QKV projection for its 2 heads (full sequence), RoPE,
causal flash-style attention, and the output rows for its heads (the
reference's permute+reshape makes output rows partition cleanly by head).

v2: bf16 operands (1 cycle/row on PE, same as f32r, but half the DMA/SBUF),
fp32 PSUM accumulation. Fused software-pipelined schedule: attention for
token-block N-1 interleaves with the QKV projection of block N so the PE
never idles (idle gaps reset the PE p-state from 2.4GHz to 1.2GHz).
Softmax denominators via a DVE running sum of the exp tiles plus a single
ones-matmul per query block instead of one ones-matmul per key block.
"""

import math
import os
import sys

for _p in ("/opt/trn_rl_repo",):
    if _p not in sys.path and os.path.isdir(_p):
        sys.path.insert(0, _p)

import ml_dtypes
import numpy as np

import concourse.bass as bass  # noqa: F401  (AP helpers)
import concourse.mybir as mybir
import concourse.tile as tile
from concourse import bacc
from concourse.bass_utils import run_bass_kernel_spmd

F32 = mybir.dt.float32
BF16 = mybir.dt.bfloat16
NPBF = ml_dtypes.bfloat16

B, T, C = 2, 2048, 2048
H, D = 16, 128
N_CORES = 8
HPC = H // N_CORES          # heads per core (2)
BT = B * T                  # 4096
KC = C // 128               # 16 contraction blocks
TB = 512                    # token block (projection AND attention)
NTB = T // TB               # 4 t-blocks per batch
SCALE = 1.0 / math.sqrt(D)

_CACHED_NC = None


def build_nc():
    nc = bacc.Bacc("TRN2", target_bir_lowering=False)

    xT = nc.dram_tensor("xT", [C, BT], BF16, kind="ExternalInput")
    wqkT = nc.dram_tensor("wqkT", [C, 4 * 128], BF16, kind="ExternalInput")
    wvT = nc.dram_tensor("wvT", [C, 2 * 128], BF16, kind="ExternalInput")
    owF = nc.dram_tensor("owF", [C, C], BF16, kind="ExternalInput")
    cosF = nc.dram_tensor("cosF", [128, T], F32, kind="ExternalInput")
    sinS = nc.dram_tensor("sinS", [128, T], F32, kind="ExternalInput")
    onesI = nc.dram_tensor("onesI", [128, 128], BF16, kind="ExternalInput")
    y = nc.dram_tensor("y", [B * HPC, 128, C], F32, kind="ExternalOutput")

    with tile.TileContext(nc) as tc:
        with tc.tile_pool(name="wpool", bufs=1) as wpool, \
             tc.tile_pool(name="xpool", bufs=8) as xpool, \
             tc.tile_pool(name="rotpool", bufs=2) as rotpool, \
             tc.tile_pool(name="vpool", bufs=2) as vpool, \
             tc.tile_pool(name="apool", bufs=2) as apool, \
             tc.tile_pool(name="epool", bufs=4) as epool, \
             tc.tile_pool(name="tpool", bufs=2) as tpool, \
             tc.tile_pool(name="dapool", bufs=2) as dapool, \
             tc.tile_pool(name="rpool", bufs=2) as rpool, \
             tc.tile_pool(name="ypool", bufs=2) as ypool, \
             tc.tile_pool(name="owpool", bufs=2) as owpool, \
             tc.tile_pool(name="flowps", bufs=3, space="PSUM") as flowps, \
             tc.tile_pool(name="attps", bufs=2, space="PSUM") as attps, \
             tc.tile_pool(name="qkps", bufs=2, space="PSUM") as qkps, \
             tc.tile_pool(name="vps", bufs=1, space="PSUM") as vps:

            twqk = wpool.tile([128, KC, 4 * 128], BF16)
            twv = wpool.tile([128, KC, 2 * 128], BF16)
            tcf = wpool.tile([128, T], F32)
            tsn = wpool.tile([128, T], F32)
            tones = wpool.tile([128, 128], BF16)
            wqkr = wqkT.rearrange("(kb p) m -> p kb m", p=128)
            wvr = wvT.rearrange("(kb p) m -> p kb m", p=128)
            owFr = owF[:, :].rearrange("(u p) j -> p u j", p=128)
            for k in range(4):
                nc.sync.dma_start(twqk[:, k, :], wqkr[:, k, :])

            # per-batch tile registries
            rots = {}   # (b, m, tb) -> [128, TB] bf16 tile
            vts = {}    # (b, tb) -> [128, 4, 256] bf16 tile
            atn = {}    # (b, h) -> [128, T] bf16 tile
            pending = []  # deferred (den matmul, rcp, atn mul) closures
            prev_mm = [None]  # deferred attnV matmul, pipelined across units
            prefetched = {}   # b -> list of preloaded owj tiles

            def flush_pending():
                while pending:
                    pending.pop(0)()

            def drain_att():
                if prev_mm[0] is not None:
                    prev_mm[0]()
                    prev_mm[0] = None
                flush_pending()

            def emit_qkv(b, tb):
                c0 = b * T + tb * TB
                ts_sl = slice(tb * TB, (tb + 1) * TB)
                first = (b == 0 and tb == 0)
                xTr = xT[:, c0:c0 + TB].rearrange("(kb p) t -> p kb t", p=128)
                xq = []
                for g in range(KC // 4):
                    xg = xpool.tile([128, 4, TB], BF16, tag="xk", name="xg")
                    nc.sync.dma_start(xg[:], xTr[:, g * 4:(g + 1) * 4, :])
                    xq.append(xg)
                    if first and g < 3:
                        # interleave remaining qk-weight blocks with x so the
                        # first chains are never starved
                        for k in range(4 * (g + 1), 4 * (g + 2)):
                            nc.sync.dma_start(twqk[:, k, :], wqkr[:, k, :])
                xk = [xq[k // 4][:, k % 4, :] for k in range(KC)]
                if first:
                    for k in range(KC):
                        nc.sync.dma_start(twv[:, k, :], wvr[:, k, :])
                    nc.sync.dma_start(tones[:], onesI[:, :])
                if b == 0:
                    # just-in-time rope table slices
                    nc.sync.dma_start(tcf[:, ts_sl], cosF[:, ts_sl])
                    nc.sync.dma_start(tsn[:, ts_sl], sinS[:, ts_sl])

                def qk_chain(m):
                    rt = rotpool.tile([128, TB], BF16, tag=f"rot{m}_{tb}",
                                      name=f"rot{m}_{tb}")
                    rots[(b, m, tb)] = rt
                    ps = qkps.tile([128, TB], F32, tag="psqk")
                    for k in range(KC):
                        nc.tensor.matmul(
                            ps[:], twqk[:, k, m * 128:(m + 1) * 128],
                            xk[k], start=(k == 0), stop=(k == KC - 1))
                    # RoPE: rows 0:64 = x1, 64:128 = x2 of this head tensor
                    qsb = tpool.tile([128, TB], F32, tag="qsb")
                    nc.scalar.copy(qsb[:], ps[:])
                    qsw = tpool.tile([128, TB], F32, tag="qsw")
                    nc.gpsimd.dma_start(qsw[0:64, :], qsb[64:128, :])
                    nc.gpsimd.dma_start(qsw[64:128, :], qsb[0:64, :])
                    pc = tpool.tile([128, TB], F32, tag="pc")
                    nc.vector.tensor_mul(out=pc[:], in0=qsb[:],
                                         in1=tcf[:, ts_sl])
                    pn = tpool.tile([128, TB], F32, tag="pn")
                    nc.gpsimd.tensor_mul(out=pn[:], in0=qsw[:],
                                         in1=tsn[:, ts_sl])
                    nc.vector.tensor_add(out=rt[:], in0=pc[:], in1=pn[:])

                vt = vpool.tile([128, 4, 2 * 128], BF16, tag=f"vt{tb}",
                                name=f"vt{tb}")
                vts[(b, tb)] = vt
                # both 128-token V chunks double-buffer inside one PSUM bank
                vbank = [vps.tile([128, 2, 2 * 128], F32, tag="psv",
                                  name="vbank") for _ in range(1)][0]

                def v_chain(ts):
                    psv = vbank[:, ts % 2, :]
                    for k in range(KC):
                        nc.tensor.matmul(
                            psv, xk[k][:, ts * 128:(ts + 1) * 128],
                            twv[:, k, :], start=(k == 0), stop=(k == KC - 1))
                    nc.vector.tensor_copy(vt[:, ts, :], psv)

                if first:
                    # x/weights still streaming in: keep PE order = DMA order
                    for m in range(4):
                        qk_chain(m)
                    for ts in range(4):
                        v_chain(ts)
                else:
                    # interleaved so the DVE casts that recycle the V PSUM
                    # bank aren't queued behind a full step of RoPE DVE work
                    for i in range(4):
                        qk_chain(i)
                        v_chain(i)

            def emit_attention(b, tb):
                for h in range(HPC):
                    if (b, h) not in atn:
                        atn[(b, h)] = apool.tile(
                            [128, T], BF16, tag=f"attnT{h}", name=f"attnT{h}")
                    at = atn[(b, h)]
                    ts_sl = slice(tb * TB, (tb + 1) * TB)
                    ns = (tb + 1) * (TB // 128)
                    ps_att = attps.tile([128, TB], F32, tag="psatt")
                    da = dapool.tile([128, TB], BF16, tag="da")
                    for si in range(ns):
                        ps_sc = flowps.tile([128, TB], F32, tag="flow")
                        et = epool.tile([128, TB], BF16, tag="et")
                        if si >= ns - TB // 128:
                            # diagonal block: cols < r fully masked; exp and
                            # scores only on [r:], ramp zero-fill [r:r+128)
                            r = si * 128 - tb * TB
                            nc.tensor.matmul(
                                ps_sc[:, r:],
                                rots[(b, 2 + h, si // 4)][
                                    :, (si % 4) * 128:(si % 4 + 1) * 128],
                                rots[(b, h, tb)][:, r:], start=True, stop=True)
                            nc.scalar.activation(
                                et[:, r:], ps_sc[:, r:],
                                mybir.ActivationFunctionType.Exp,
                                scale=SCALE)
                            nc.gpsimd.affine_select(
                                out=et[:, r:r + 128], in_=et[:, r:r + 128],
                                compare_op=mybir.AluOpType.is_ge,
                                fill=0.0, base=0,
                                pattern=[[1, 128]],
                                channel_multiplier=-1)
                            if r > 0:
                                nc.gpsimd.memset(et[:, :r], 0.0)
                        else:
                            nc.tensor.matmul(
                                ps_sc[:],
                                rots[(b, 2 + h, si // 4)][
                                    :, (si % 4) * 128:(si % 4 + 1) * 128],
                                rots[(b, h, tb)][:], start=True, stop=True)
                            nc.scalar.activation(
                                et[:], ps_sc[:],
                                mybir.ActivationFunctionType.Exp,
                                scale=SCALE)
                        # denominator running sum on DVE (frees PE cycles)
                        if si == 0:
                            nc.vector.tensor_copy(da[:], et[:])
                        else:
                            nc.vector.tensor_add(out=da[:], in0=da[:],
                                                 in1=et[:])
                        # 1-stage pipeline ACROSS units: consume the PREVIOUS
                        # et (possibly the last one of the previous unit) so
                        # the in-order PE always has a scores-mm queued ahead
                        # of each exp-dependent attnV-mm
                        if prev_mm[0] is not None:
                            prev_mm[0]()
                        if si == 1:
                            # previous unit's epilogue (den/rcp/mul), now that
                            # its last attnV was emitted at si==0
                            flush_pending()

                        def attnv(ep=et, p=si, ps_att=ps_att, b=b, h=h,
                                  last=(si == ns - 1)):
                            nc.tensor.matmul(
                                ps_att[:],
                                vts[(b, p // 4)][:, p % 4,
                                                 h * 128:(h + 1) * 128],
                                ep[:], start=(p == 0), stop=last)
                        prev_mm[0] = attnv

                    def epilogue(ps_att=ps_att, da=da, at=at, ts_sl=ts_sl):
                        ps_den = flowps.tile([128, TB], F32, tag="flow")
                        nc.tensor.matmul(ps_den[:], tones[:], da[:],
                                         start=True, stop=True)
                        rcp = rpool.tile([128, TB], F32, tag="rcp")
                        nc.vector.reciprocal_approx_fast(out=rcp[:],
                                                         in_=ps_den[:])
                        nc.vector.tensor_mul(out=at[:, ts_sl],
                                             in0=ps_att[:], in1=rcp[:])
                    pending.append(epilogue)

            OJ = 512

            def prefetch_proj(b):
                tiles = []
                for jb in range(2):
                    owj = owpool.tile([128, KC, OJ], BF16, tag="owj")
                    nc.sync.dma_start(
                        owj[:], owFr[:, :, jb * OJ:(jb + 1) * OJ])
                    tiles.append(owj)
                prefetched[b] = tiles

            def emit_proj(b):
                drain_att()
                for jb in range(C // OJ):
                    if jb < 2:
                        owj = prefetched[b][jb]
                    else:
                        owj = owpool.tile([128, KC, OJ], BF16, tag="owj")
                        nc.sync.dma_start(
                            owj[:], owFr[:, :, jb * OJ:(jb + 1) * OJ])
                    for h in range(HPC):
                        av = atn[(b, h)][:].rearrange("p (a u) -> p a u", u=16)
                        psy = flowps.tile([128, OJ], F32, tag="flow")
                        for u in range(KC):
                            nc.tensor.matmul(
                                psy[:], av[:, :, u], owj[:, u, :],
                                start=(u == 0), stop=(u == KC - 1))
                        ys = ypool.tile([128, OJ], F32, tag="ys")
                        nc.scalar.copy(ys[:], psy[:])
                        nc.sync.dma_start(
                            y[b * HPC + h, :, jb * OJ:(jb + 1) * OJ], ys[:])

            # software pipeline: qkv(step) overlaps attention(step-1);
            # out-proj(b) emitted right after attention (b, NTB-1)
            steps = [(s // NTB, s % NTB) for s in range(B * NTB)]
            for s in range(B * NTB + 1):
                if s < B * NTB:
                    emit_qkv(*steps[s])
                if s >= 1:
                    bq, tbq = steps[s - 1]
                    if tbq == NTB - 1:
                        prefetch_proj(bq)
                    emit_attention(bq, tbq)
                    if tbq == NTB - 1:
                        emit_proj(bq)
    nc.compile()
    return nc


def _get_nc():
    global _CACHED_NC
    if _CACHED_NC is None:
        _CACHED_NC = build_nc()
    return _CACHED_NC


def _rope_tables():
    pos = np.arange(T, dtype=np.float64)[:, None]
    div = np.exp(np.arange(0, D, 2, dtype=np.float64) *
                 (-math.log(10000.0) / D))
    ang = pos * div  # [T, 64]
    sinT = np.sin(ang).T.astype(np.float32)  # [64, T]
    cosT = np.cos(ang).T.astype(np.float32)
    cosF = np.ascontiguousarray(np.concatenate([cosT, cosT], axis=0))
    sinS = np.ascontiguousarray(np.concatenate([-sinT, sinT], axis=0))
    return cosF, sinS


def make_in_maps(x, qkv_w, out_w):
    xT = np.ascontiguousarray(x.reshape(BT, C).T.astype(NPBF))
    owF = np.ascontiguousarray(out_w.T.astype(NPBF))
    cosF, sinS = _rope_tables()
    ones = np.ones((128, 128), dtype=NPBF)
    in_maps = []
    for c in range(N_CORES):
        h0, h1 = 2 * c, 2 * c + 1
        wqk = np.concatenate([
            qkv_w[h0 * D:(h0 + 1) * D],
            qkv_w[h1 * D:(h1 + 1) * D],
            qkv_w[C + h0 * D:C + (h0 + 1) * D],
            qkv_w[C + h1 * D:C + (h1 + 1) * D],
        ], axis=0)                       # [512, 2048]
        wv = np.concatenate([
            qkv_w[2 * C + h0 * D:2 * C + (h0 + 1) * D],
            qkv_w[2 * C + h1 * D:2 * C + (h1 + 1) * D],
        ], axis=0)                       # [256, 2048]
        in_maps.append({
            "xT": xT,
            "wqkT": np.ascontiguousarray(wqk.T.astype(NPBF)),
            "wvT": np.ascontiguousarray(wv.T.astype(NPBF)),
            "owF": owF,
            "cosF": cosF,
            "sinS": sinS,
            "onesI": ones,
        })
    return in_maps


def kernel(x, qkv_w, out_w, _trace=False, _trace_kwargs=None):
    x = np.asarray(x, dtype=np.float32)
    qkv_w = np.asarray(qkv_w, dtype=np.float32)
    out_w = np.asarray(out_w, dtype=np.float32)
    nc = _get_nc()
    in_maps = make_in_maps(x, qkv_w, out_w)
    kwargs = {}
    if _trace:
        kwargs["trace"] = True
        if _trace_kwargs:
            kwargs.update(_trace_kwargs)
    res = run_bass_kernel_spmd(nc, in_maps, core_ids=list(range(N_CORES)),
                               **kwargs)
    out = np.empty((B, T, C), dtype=np.float32)
    for c in range(N_CORES):
        yc = res.results[c]["y"]  # [B*HPC, 128, C]
        for b in range(B):
            for hl in range(HPC):
                hg = HPC * c + hl
                out[b, hg * 128:(hg + 1) * 128] = yc[b * HPC + hl]
    if _trace:
        return out, res
    return out


# revision 18
# speedup vs baseline: 1.1721x; 1.1721x over previous
"""Trainium2 Bass kernel for causal multi-head attention with RoPE.

Sharding: tensor-parallel over heads. 16 heads / 8 cores = 2 heads per core.
Each core computes QKV projection for its 2 heads (full sequence), RoPE,
causal flash-style attention, and the output rows for its heads (the
reference's permute+reshape makes output rows partition cleanly by head).

v2: bf16 operands (1 cycle/row on PE, same as f32r, but half the DMA/SBUF),
fp32 PSUM accumulation. Fused software-pipelined schedule: attention for
token-block N-1 interleaves with the QKV projection of block N so the PE
never idles (idle gaps reset the PE p-state from 2.4GHz to 1.2GHz).
Softmax denominators via a DVE running sum of the exp tiles plus a single
ones-matmul per query block instead of one ones-matmul per key block.
"""

import math
import os
import sys

for _p in ("/opt/trn_rl_repo",):
    if _p not in sys.path and os.path.isdir(_p):
        sys.path.insert(0, _p)

import ml_dtypes
import numpy as np

import concourse.bass as bass  # noqa: F401  (AP helpers)
import concourse.mybir as mybir
import concourse.tile as tile
from concourse import bacc
from concourse.bass_utils import run_bass_kernel_spmd

F32 = mybir.dt.float32
BF16 = mybir.dt.bfloat16
NPBF = ml_dtypes.bfloat16

B, T, C = 2, 2048, 2048
H, D = 16, 128
N_CORES = 8
HPC = H // N_CORES          # heads per core (2)
BT = B * T                  # 4096
KC = C // 128               # 16 contraction blocks
TB = 512                    # token block (projection AND attention)
NTB = T // TB               # 4 t-blocks per batch
SCALE = 1.0 / math.sqrt(D)

_CACHED_NC = None


def build_nc():
    nc = bacc.Bacc("TRN2", target_bir_lowering=False)

    xT = nc.dram_tensor("xT", [C, BT], BF16, kind="ExternalInput")
    wqkT = nc.dram_tensor("wqkT", [C, 4 * 128], BF16, kind="ExternalInput")
    wvT = nc.dram_tensor("wvT", [C, 2 * 128], BF16, kind="ExternalInput")
    owF = nc.dram_tensor("owF", [C, C], BF16, kind="ExternalInput")
    cosF = nc.dram_tensor("cosF", [128, T], F32, kind="ExternalInput")
    sinS = nc.dram_tensor("sinS", [128, T], F32, kind="ExternalInput")
    onesI = nc.dram_tensor("onesI", [128, 128], BF16, kind="ExternalInput")
    y = nc.dram_tensor("y", [B * HPC, 128, C], F32, kind="ExternalOutput")

    with tile.TileContext(nc) as tc:
        with tc.tile_pool(name="wpool", bufs=1) as wpool, \
             tc.tile_pool(name="xpool", bufs=8) as xpool, \
             tc.tile_pool(name="rotpool", bufs=2) as rotpool, \
             tc.tile_pool(name="vpool", bufs=2) as vpool, \
             tc.tile_pool(name="apool", bufs=2) as apool, \
             tc.tile_pool(name="epool", bufs=4) as epool, \
             tc.tile_pool(name="tpool", bufs=2) as tpool, \
             tc.tile_pool(name="dapool", bufs=2) as dapool, \
             tc.tile_pool(name="rpool", bufs=2) as rpool, \
             tc.tile_pool(name="ypool", bufs=2) as ypool, \
             tc.tile_pool(name="owpool", bufs=2) as owpool, \
             tc.tile_pool(name="flowps", bufs=3, space="PSUM") as flowps, \
             tc.tile_pool(name="attps", bufs=2, space="PSUM") as attps, \
             tc.tile_pool(name="qkps", bufs=2, space="PSUM") as qkps, \
             tc.tile_pool(name="vps", bufs=1, space="PSUM") as vps:

            twqk = wpool.tile([128, KC, 4 * 128], BF16)
            twv = wpool.tile([128, KC, 2 * 128], BF16)
            tcf = wpool.tile([128, T], F32)
            tsn = wpool.tile([128, T], F32)
            tones = wpool.tile([128, 128], BF16)
            wqkr = wqkT.rearrange("(kb p) m -> p kb m", p=128)
            wvr = wvT.rearrange("(kb p) m -> p kb m", p=128)
            owFr = owF[:, :].rearrange("(u p) j -> p u j", p=128)
            for k in range(4):
                nc.sync.dma_start(twqk[:, k, :], wqkr[:, k, :])

            # per-batch tile registries
            rots = {}   # (b, m, tb) -> [128, TB] bf16 tile
            vts = {}    # (b, tb) -> [128, 4, 256] bf16 tile
            atn = {}    # (b, h) -> [128, T] bf16 tile
            pending = []  # deferred (den matmul, rcp, atn mul) closures
            prev_mm = [None]  # deferred attnV matmul, pipelined across units
            prefetched = {}   # b -> list of preloaded owj tiles

            def flush_pending():
                while pending:
                    pending.pop(0)()

            def drain_att():
                if prev_mm[0] is not None:
                    prev_mm[0]()
                    prev_mm[0] = None
                flush_pending()

            def emit_qkv(b, tb):
                c0 = b * T + tb * TB
                ts_sl = slice(tb * TB, (tb + 1) * TB)
                first = (b == 0 and tb == 0)
                xTr = xT[:, c0:c0 + TB].rearrange("(kb p) t -> p kb t", p=128)
                xq = []
                for g in range(KC // 4):
                    xg = xpool.tile([128, 4, TB], BF16, tag="xk", name="xg")
                    nc.sync.dma_start(xg[:], xTr[:, g * 4:(g + 1) * 4, :])
                    xq.append(xg)
                    if first and g < 3:
                        # interleave remaining qk-weight blocks with x so the
                        # first chains are never starved
                        for k in range(4 * (g + 1), 4 * (g + 2)):
                            nc.sync.dma_start(twqk[:, k, :], wqkr[:, k, :])
                xk = [xq[k // 4][:, k % 4, :] for k in range(KC)]
                if first:
                    for k in range(KC):
                        nc.sync.dma_start(twv[:, k, :], wvr[:, k, :])
                    nc.sync.dma_start(tones[:], onesI[:, :])
                if b == 0:
                    # just-in-time rope table slices
                    nc.sync.dma_start(tcf[:, ts_sl], cosF[:, ts_sl])
                    nc.sync.dma_start(tsn[:, ts_sl], sinS[:, ts_sl])

                def qk_chain(m):
                    rt = rotpool.tile([128, TB], BF16, tag=f"rot{m}_{tb}",
                                      name=f"rot{m}_{tb}")
                    rots[(b, m, tb)] = rt
                    ps = qkps.tile([128, TB], F32, tag="psqk")
                    for k in range(KC):
                        nc.tensor.matmul(
                            ps[:], twqk[:, k, m * 128:(m + 1) * 128],
                            xk[k], start=(k == 0), stop=(k == KC - 1))
                    # RoPE: rows 0:64 = x1, 64:128 = x2 of this head tensor
                    qsb = tpool.tile([128, TB], F32, tag="qsb")
                    nc.scalar.copy(qsb[:], ps[:])
                    qsw = tpool.tile([128, TB], F32, tag="qsw")
                    nc.gpsimd.dma_start(qsw[0:64, :], qsb[64:128, :])
                    nc.gpsimd.dma_start(qsw[64:128, :], qsb[0:64, :])
                    pc = tpool.tile([128, TB], F32, tag="pc")
                    nc.vector.tensor_mul(out=pc[:], in0=qsb[:],
                                         in1=tcf[:, ts_sl])
                    pn = tpool.tile([128, TB], F32, tag="pn")
                    nc.gpsimd.tensor_mul(out=pn[:], in0=qsw[:],
                                         in1=tsn[:, ts_sl])
                    nc.vector.tensor_add(out=rt[:], in0=pc[:], in1=pn[:])

                vt = vpool.tile([128, 4, 2 * 128], BF16, tag=f"vt{tb}",
                                name=f"vt{tb}")
                vts[(b, tb)] = vt
                # both 128-token V chunks double-buffer inside one PSUM bank
                vbank = [vps.tile([128, 2, 2 * 128], F32, tag="psv",
                                  name="vbank") for _ in range(1)][0]

                def v_chain(ts):
                    psv = vbank[:, ts % 2, :]
                    for k in range(KC):
                        nc.tensor.matmul(
                            psv, xk[k][:, ts * 128:(ts + 1) * 128],
                            twv[:, k, :], start=(k == 0), stop=(k == KC - 1))
                    nc.vector.tensor_copy(vt[:, ts, :], psv)

                if first:
                    # x/weights still streaming in: keep PE order = DMA order
                    for m in range(4):
                        qk_chain(m)
                    for ts in range(4):
                        v_chain(ts)
                else:
                    # interleaved so the DVE casts that recycle the V PSUM
                    # bank aren't queued behind a full step of RoPE DVE work
                    for i in range(4):
                        qk_chain(i)
                        v_chain(i)

            def emit_attention(b, tb):
                for h in range(HPC):
                    if (b, h) not in atn:
                        atn[(b, h)] = apool.tile(
                            [128, T], BF16, tag=f"attnT{h}", name=f"attnT{h}")
                    at = atn[(b, h)]
                    ts_sl = slice(tb * TB, (tb + 1) * TB)
                    ns = (tb + 1) * (TB // 128)
                    ps_att = attps.tile([128, TB], F32, tag="psatt")
                    da = dapool.tile([128, TB], BF16, tag="da")
                    et_last = [None]
                    for si in range(ns):
                        ps_sc = flowps.tile([128, TB], F32, tag="flow")
                        et = epool.tile([128, TB], BF16, tag="et")
                        if si >= ns - TB // 128:
                            # diagonal block: cols < r fully masked; exp and
                            # scores only on [r:], ramp zero-fill [r:r+128)
                            r = si * 128 - tb * TB
                            nc.tensor.matmul(
                                ps_sc[:, r:],
                                rots[(b, 2 + h, si // 4)][
                                    :, (si % 4) * 128:(si % 4 + 1) * 128],
                                rots[(b, h, tb)][:, r:], start=True, stop=True)
                            nc.scalar.activation(
                                et[:, r:], ps_sc[:, r:],
                                mybir.ActivationFunctionType.Exp,
                                scale=SCALE)
                            nc.gpsimd.affine_select(
                                out=et[:, r:r + 128], in_=et[:, r:r + 128],
                                compare_op=mybir.AluOpType.is_ge,
                                fill=0.0, base=0,
                                pattern=[[1, 128]],
                                channel_multiplier=-1)
                            if r > 0:
                                nc.gpsimd.memset(et[:, :r], 0.0)
                        else:
                            nc.tensor.matmul(
                                ps_sc[:],
                                rots[(b, 2 + h, si // 4)][
                                    :, (si % 4) * 128:(si % 4 + 1) * 128],
                                rots[(b, h, tb)][:], start=True, stop=True)
                            nc.scalar.activation(
                                et[:], ps_sc[:],
                                mybir.ActivationFunctionType.Exp,
                                scale=SCALE)
                        # denominator running sum on DVE (frees PE cycles);
                        # the last block is folded in by a second accumulating
                        # ones-matmul instead, cutting the serial DVE tail
                        if si == 0:
                            nc.vector.tensor_copy(da[:], et[:])
                        elif si < ns - 1:
                            nc.vector.tensor_add(out=da[:], in0=da[:],
                                                 in1=et[:])
                        else:
                            et_last[0] = et
                        # 1-stage pipeline ACROSS units: consume the PREVIOUS
                        # et (possibly the last one of the previous unit) so
                        # the in-order PE always has a scores-mm queued ahead
                        # of each exp-dependent attnV-mm
                        if prev_mm[0] is not None:
                            prev_mm[0]()
                        if si == 1:
                            # previous unit's epilogue (den/rcp/mul), now that
                            # its last attnV was emitted at si==0
                            flush_pending()

                        def attnv(ep=et, p=si, ps_att=ps_att, b=b, h=h,
                                  last=(si == ns - 1)):
                            nc.tensor.matmul(
                                ps_att[:],
                                vts[(b, p // 4)][:, p % 4,
                                                 h * 128:(h + 1) * 128],
                                ep[:], start=(p == 0), stop=last)
                        prev_mm[0] = attnv

                    def epilogue(ps_att=ps_att, da=da, at=at, ts_sl=ts_sl,
                                 el=et_last, ns=ns):
                        ps_den = flowps.tile([128, TB], F32, tag="flow")
                        nc.tensor.matmul(ps_den[:], tones[:], da[:],
                                         start=True, stop=(ns == 1))
                        if ns > 1:
                            nc.tensor.matmul(ps_den[:], tones[:], el[0][:],
                                             start=False, stop=True)
                        rcp = rpool.tile([128, TB], F32, tag="rcp")
                        nc.vector.reciprocal_approx_fast(out=rcp[:],
                                                         in_=ps_den[:])
                        nc.vector.tensor_mul(out=at[:, ts_sl],
                                             in0=ps_att[:], in1=rcp[:])
                    pending.append(epilogue)

            OJ = 512

            def prefetch_proj(b):
                tiles = []
                for jb in range(2):
                    owj = owpool.tile([128, KC, OJ], BF16, tag="owj")
                    nc.sync.dma_start(
                        owj[:], owFr[:, :, jb * OJ:(jb + 1) * OJ])
                    tiles.append(owj)
                prefetched[b] = tiles

            def emit_proj(b):
                drain_att()
                for jb in range(C // OJ):
                    if jb < 2:
                        owj = prefetched[b][jb]
                    else:
                        owj = owpool.tile([128, KC, OJ], BF16, tag="owj")
                        nc.sync.dma_start(
                            owj[:], owFr[:, :, jb * OJ:(jb + 1) * OJ])
                    for h in range(HPC):
                        av = atn[(b, h)][:].rearrange("p (a u) -> p a u", u=16)
                        psy = flowps.tile([128, OJ], F32, tag="flow")
                        for u in range(KC):
                            nc.tensor.matmul(
                                psy[:], av[:, :, u], owj[:, u, :],
                                start=(u == 0), stop=(u == KC - 1))
                        ys = ypool.tile([128, OJ], F32, tag="ys")
                        nc.scalar.copy(ys[:], psy[:])
                        nc.sync.dma_start(
                            y[b * HPC + h, :, jb * OJ:(jb + 1) * OJ], ys[:])

            # software pipeline: qkv(step) overlaps attention(step-1);
            # out-proj(b) emitted right after attention (b, NTB-1)
            # software pipeline, attention(s-1) EMITTED BEFORE qkv(s): every
            # engine's in-order queue then serves the attention work (which
            # the PE needs now) before the next block's rope/cast work
            steps = [(s // NTB, s % NTB) for s in range(B * NTB)]
            for s in range(B * NTB + 1):
                if s >= 1:
                    bq, tbq = steps[s - 1]
                    if tbq == NTB - 1:
                        prefetch_proj(bq)
                    emit_attention(bq, tbq)
                    if tbq == NTB - 1:
                        emit_proj(bq)
                if s < B * NTB:
                    emit_qkv(*steps[s])
    nc.compile()
    return nc


def _get_nc():
    global _CACHED_NC
    if _CACHED_NC is None:
        _CACHED_NC = build_nc()
    return _CACHED_NC


def _rope_tables():
    pos = np.arange(T, dtype=np.float64)[:, None]
    div = np.exp(np.arange(0, D, 2, dtype=np.float64) *
                 (-math.log(10000.0) / D))
    ang = pos * div  # [T, 64]
    sinT = np.sin(ang).T.astype(np.float32)  # [64, T]
    cosT = np.cos(ang).T.astype(np.float32)
    cosF = np.ascontiguousarray(np.concatenate([cosT, cosT], axis=0))
    sinS = np.ascontiguousarray(np.concatenate([-sinT, sinT], axis=0))
    return cosF, sinS


def make_in_maps(x, qkv_w, out_w):
    xT = np.ascontiguousarray(x.reshape(BT, C).T.astype(NPBF))
    owF = np.ascontiguousarray(out_w.T.astype(NPBF))
    cosF, sinS = _rope_tables()
    ones = np.ones((128, 128), dtype=NPBF)
    in_maps = []
    for c in range(N_CORES):
        h0, h1 = 2 * c, 2 * c + 1
        wqk = np.concatenate([
            qkv_w[h0 * D:(h0 + 1) * D],
            qkv_w[h1 * D:(h1 + 1) * D],
            qkv_w[C + h0 * D:C + (h0 + 1) * D],
            qkv_w[C + h1 * D:C + (h1 + 1) * D],
        ], axis=0)                       # [512, 2048]
        wv = np.concatenate([
            qkv_w[2 * C + h0 * D:2 * C + (h0 + 1) * D],
            qkv_w[2 * C + h1 * D:2 * C + (h1 + 1) * D],
        ], axis=0)                       # [256, 2048]
        in_maps.append({
            "xT": xT,
            "wqkT": np.ascontiguousarray(wqk.T.astype(NPBF)),
            "wvT": np.ascontiguousarray(wv.T.astype(NPBF)),
            "owF": owF,
            "cosF": cosF,
            "sinS": sinS,
            "onesI": ones,
        })
    return in_maps


def kernel(x, qkv_w, out_w, _trace=False, _trace_kwargs=None):
    x = np.asarray(x, dtype=np.float32)
    qkv_w = np.asarray(qkv_w, dtype=np.float32)
    out_w = np.asarray(out_w, dtype=np.float32)
    nc = _get_nc()
    in_maps = make_in_maps(x, qkv_w, out_w)
    kwargs = {}
    if _trace:
        kwargs["trace"] = True
        if _trace_kwargs:
            kwargs.update(_trace_kwargs)
    res = run_bass_kernel_spmd(nc, in_maps, core_ids=list(range(N_CORES)),
                               **kwargs)
    out = np.empty((B, T, C), dtype=np.float32)
    for c in range(N_CORES):
        yc = res.results[c]["y"]  # [B*HPC, 128, C]
        for b in range(B):
            for hl in range(HPC):
                hg = HPC * c + hl
                out[b, hg * 128:(hg + 1) * 128] = yc[b * HPC + hl]
    if _trace:
        return out, res
    return out


# revision 25
# speedup vs baseline: 1.1763x; 1.0036x over previous
"""Trainium2 Bass kernel for causal multi-head attention with RoPE.

Sharding: tensor-parallel over heads. 16 heads / 8 cores = 2 heads per core.
Each core computes QKV projection for its 2 heads (full sequence), RoPE,
causal flash-style attention, and the output rows for its heads (the
reference's permute+reshape makes output rows partition cleanly by head).

v2: bf16 operands (1 cycle/row on PE, same as f32r, but half the DMA/SBUF),
fp32 PSUM accumulation. Fused software-pipelined schedule: attention for
token-block N-1 interleaves with the QKV projection of block N so the PE
never idles (idle gaps reset the PE p-state from 2.4GHz to 1.2GHz).
Softmax denominators via a DVE running sum of the exp tiles plus a single
ones-matmul per query block instead of one ones-matmul per key block.
"""

import math
import os
import sys

for _p in ("/opt/trn_rl_repo",):
    if _p not in sys.path and os.path.isdir(_p):
        sys.path.insert(0, _p)

import ml_dtypes
import numpy as np

import concourse.bass as bass  # noqa: F401  (AP helpers)
import concourse.mybir as mybir
import concourse.tile as tile
from concourse import bacc
from concourse.bass_utils import run_bass_kernel_spmd

F32 = mybir.dt.float32
BF16 = mybir.dt.bfloat16
NPBF = ml_dtypes.bfloat16

B, T, C = 2, 2048, 2048
H, D = 16, 128
N_CORES = 8
HPC = H // N_CORES          # heads per core (2)
BT = B * T                  # 4096
KC = C // 128               # 16 contraction blocks
TB = 512                    # token block (projection AND attention)
NTB = T // TB               # 4 t-blocks per batch
SCALE = 1.0 / math.sqrt(D)

_CACHED_NC = None


def build_nc():
    nc = bacc.Bacc("TRN2", target_bir_lowering=False)

    xT = nc.dram_tensor("xT", [C, BT], BF16, kind="ExternalInput")
    wqkT = nc.dram_tensor("wqkT", [C, 4 * 128], BF16, kind="ExternalInput")
    wvT = nc.dram_tensor("wvT", [C, 2 * 128], BF16, kind="ExternalInput")
    owF = nc.dram_tensor("owF", [C, C], BF16, kind="ExternalInput")
    cosF = nc.dram_tensor("cosF", [128, T], F32, kind="ExternalInput")
    sinS = nc.dram_tensor("sinS", [128, T], F32, kind="ExternalInput")
    onesI = nc.dram_tensor("onesI", [128, 128], BF16, kind="ExternalInput")
    triM = nc.dram_tensor("triM", [128, 128], BF16, kind="ExternalInput")
    y = nc.dram_tensor("y", [B * HPC, 128, C], F32, kind="ExternalOutput")

    with tile.TileContext(nc) as tc:
        with tc.tile_pool(name="wpool", bufs=1) as wpool, \
             tc.tile_pool(name="xpool", bufs=8) as xpool, \
             tc.tile_pool(name="rotpool", bufs=2) as rotpool, \
             tc.tile_pool(name="vpool", bufs=2) as vpool, \
             tc.tile_pool(name="apool", bufs=2) as apool, \
             tc.tile_pool(name="epool", bufs=4) as epool, \
             tc.tile_pool(name="tpool", bufs=2) as tpool, \
             tc.tile_pool(name="dapool", bufs=2) as dapool, \
             tc.tile_pool(name="rpool", bufs=2) as rpool, \
             tc.tile_pool(name="ypool", bufs=2) as ypool, \
             tc.tile_pool(name="owpool", bufs=2) as owpool, \
             tc.tile_pool(name="flowps", bufs=3, space="PSUM") as flowps, \
             tc.tile_pool(name="attps", bufs=2, space="PSUM") as attps, \
             tc.tile_pool(name="qkps", bufs=2, space="PSUM") as qkps, \
             tc.tile_pool(name="vps", bufs=1, space="PSUM") as vps:

            twqk = wpool.tile([128, KC, 4 * 128], BF16)
            twv = wpool.tile([128, KC, 2 * 128], BF16)
            tcf = wpool.tile([128, T], F32)
            tsn = wpool.tile([128, T], F32)
            tones = wpool.tile([128, 128], BF16)
            ttri = wpool.tile([128, 128], BF16)
            wqkr = wqkT.rearrange("(kb p) m -> p kb m", p=128)
            wvr = wvT.rearrange("(kb p) m -> p kb m", p=128)
            owFr = owF[:, :].rearrange("(u p) j -> p u j", p=128)
            for k in range(4):
                nc.sync.dma_start(twqk[:, k, :], wqkr[:, k, :])

            # per-batch tile registries
            rots = {}   # (b, m, tb) -> [128, TB] bf16 tile
            vts = {}    # (b, tb) -> [128, 4, 256] bf16 tile
            atn = {}    # (b, h) -> [128, T] bf16 tile
            pending = []  # deferred (den matmul, rcp, atn mul) closures
            prev_mm = [None]  # deferred attnV matmul, pipelined across units
            prefetched = {}   # b -> list of preloaded owj tiles

            def flush_pending():
                while pending:
                    pending.pop(0)()

            def drain_att():
                if prev_mm[0] is not None:
                    prev_mm[0]()
                    prev_mm[0] = None
                flush_pending()

            def emit_qkv(b, tb):
                c0 = b * T + tb * TB
                ts_sl = slice(tb * TB, (tb + 1) * TB)
                first = (b == 0 and tb == 0)
                xTr = xT[:, c0:c0 + TB].rearrange("(kb p) t -> p kb t", p=128)
                xq = []
                for g in range(KC // 4):
                    xg = xpool.tile([128, 4, TB], BF16, tag="xk", name="xg")
                    nc.sync.dma_start(xg[:], xTr[:, g * 4:(g + 1) * 4, :])
                    xq.append(xg)
                    if first and g < 3:
                        # interleave remaining qk-weight blocks with x so the
                        # first chains are never starved
                        for k in range(4 * (g + 1), 4 * (g + 2)):
                            nc.sync.dma_start(twqk[:, k, :], wqkr[:, k, :])
                xk = [xq[k // 4][:, k % 4, :] for k in range(KC)]
                if first:
                    for k in range(KC):
                        nc.sync.dma_start(twv[:, k, :], wvr[:, k, :])
                    nc.sync.dma_start(tones[:], onesI[:, :])
                    nc.sync.dma_start(ttri[:], triM[:, :])
                if b == 0:
                    # just-in-time rope table slices
                    nc.sync.dma_start(tcf[:, ts_sl], cosF[:, ts_sl])
                    nc.sync.dma_start(tsn[:, ts_sl], sinS[:, ts_sl])

                def qk_chain(m):
                    rt = rotpool.tile([128, TB], BF16, tag=f"rot{m}_{tb}",
                                      name=f"rot{m}_{tb}")
                    rots[(b, m, tb)] = rt
                    ps = qkps.tile([128, TB], F32, tag="psqk")
                    for k in range(KC):
                        nc.tensor.matmul(
                            ps[:], twqk[:, k, m * 128:(m + 1) * 128],
                            xk[k], start=(k == 0), stop=(k == KC - 1))
                    # RoPE: rows 0:64 = x1, 64:128 = x2 of this head tensor
                    qsb = tpool.tile([128, TB], F32, tag="qsb")
                    nc.scalar.copy(qsb[:], ps[:])
                    qsw = tpool.tile([128, TB], F32, tag="qsw")
                    nc.gpsimd.dma_start(qsw[0:64, :], qsb[64:128, :])
                    nc.gpsimd.dma_start(qsw[64:128, :], qsb[0:64, :])
                    pc = tpool.tile([128, TB], F32, tag="pc")
                    nc.vector.tensor_mul(out=pc[:], in0=qsb[:],
                                         in1=tcf[:, ts_sl])
                    pn = tpool.tile([128, TB], F32, tag="pn")
                    nc.gpsimd.tensor_mul(out=pn[:], in0=qsw[:],
                                         in1=tsn[:, ts_sl])
                    nc.vector.tensor_add(out=rt[:], in0=pc[:], in1=pn[:])

                vt = vpool.tile([128, 4, 2 * 128], BF16, tag=f"vt{tb}",
                                name=f"vt{tb}")
                vts[(b, tb)] = vt
                # both 128-token V chunks double-buffer inside one PSUM bank
                vbank = [vps.tile([128, 2, 2 * 128], F32, tag="psv",
                                  name="vbank") for _ in range(1)][0]

                def v_chain(ts):
                    psv = vbank[:, ts % 2, :]
                    for k in range(KC):
                        nc.tensor.matmul(
                            psv, xk[k][:, ts * 128:(ts + 1) * 128],
                            twv[:, k, :], start=(k == 0), stop=(k == KC - 1))
                    nc.vector.tensor_copy(vt[:, ts, :], psv)

                if first:
                    # x/weights still streaming in: keep PE order = DMA order
                    for m in range(4):
                        qk_chain(m)
                    for ts in range(4):
                        v_chain(ts)
                else:
                    # interleaved so the DVE casts that recycle the V PSUM
                    # bank aren't queued behind a full step of RoPE DVE work
                    for i in range(4):
                        qk_chain(i)
                        v_chain(i)

            def emit_attention(b, tb):
                for h in range(HPC):
                    if (b, h) not in atn:
                        atn[(b, h)] = apool.tile(
                            [128, T], BF16, tag=f"attnT{h}", name=f"attnT{h}")
                    at = atn[(b, h)]
                    ts_sl = slice(tb * TB, (tb + 1) * TB)
                    ns = (tb + 1) * (TB // 128)
                    ps_att = attps.tile([128, TB], F32, tag="psatt")
                    da = dapool.tile([128, TB], BF16, tag="da")
                    et_last = [None]
                    for si in range(ns):
                        ps_sc = flowps.tile([128, TB], F32, tag="flow")
                        et = epool.tile([128, TB], BF16, tag="et")
                        diag = si >= ns - TB // 128
                        # cols < r of a diagonal block are fully masked: they
                        # are simply never computed nor read downstream
                        r = si * 128 - tb * TB if diag else 0
                        nc.tensor.matmul(
                            ps_sc[:, r:],
                            rots[(b, 2 + h, si // 4)][
                                :, (si % 4) * 128:(si % 4 + 1) * 128],
                            rots[(b, h, tb)][:, r:], start=True, stop=True)
                        nc.scalar.activation(
                            et[:, r:], ps_sc[:, r:],
                            mybir.ActivationFunctionType.Exp,
                            scale=SCALE)
                        if diag:
                            # triangular ramp masking on DVE (Pool's in-order
                            # queue is full of next-block rope work)
                            nc.vector.tensor_mul(
                                out=et[:, r:r + 128], in0=et[:, r:r + 128],
                                in1=ttri[:, :])
                        # denominator running sum on DVE (frees PE cycles);
                        # the last block is folded in by a second accumulating
                        # ones-matmul instead, cutting the serial DVE tail
                        if si == 0:
                            nc.vector.tensor_copy(da[:], et[:])
                        elif si < ns - 1:
                            nc.vector.tensor_add(out=da[:, r:],
                                                 in0=da[:, r:],
                                                 in1=et[:, r:])
                        else:
                            et_last[0] = et
                        # 1-stage pipeline ACROSS units: consume the PREVIOUS
                        # et (possibly the last one of the previous unit) so
                        # the in-order PE always has a scores-mm queued ahead
                        # of each exp-dependent attnV-mm
                        if prev_mm[0] is not None:
                            prev_mm[0]()
                        if si == 1:
                            # previous unit's epilogue (den/rcp/mul), now that
                            # its last attnV was emitted at si==0
                            flush_pending()

                        def attnv(ep=et, p=si, ps_att=ps_att, b=b, h=h, r=r,
                                  last=(si == ns - 1)):
                            nc.tensor.matmul(
                                ps_att[:, r:],
                                vts[(b, p // 4)][:, p % 4,
                                                 h * 128:(h + 1) * 128],
                                ep[:, r:], start=(p == 0), stop=last,
                                skip_group_check=(r > 0))
                        prev_mm[0] = attnv

                    def epilogue(ps_att=ps_att, da=da, at=at, ts_sl=ts_sl,
                                 el=et_last):
                        # last diag block only covers cols [TB-128:]
                        rl = TB - 128
                        ps_den = flowps.tile([128, TB], F32, tag="flow")
                        nc.tensor.matmul(ps_den[:], tones[:], da[:],
                                         start=True, stop=False,
                                         skip_group_check=True)
                        nc.tensor.matmul(ps_den[:, rl:], tones[:],
                                         el[0][:, rl:], start=False,
                                         stop=True, skip_group_check=True)
                        rcp = rpool.tile([128, TB], F32, tag="rcp")
                        nc.vector.reciprocal_approx_fast(out=rcp[:],
                                                         in_=ps_den[:])
                        nc.vector.tensor_mul(out=at[:, ts_sl],
                                             in0=ps_att[:], in1=rcp[:])
                    pending.append(epilogue)

            OJ = 512

            def prefetch_proj(b):
                tiles = []
                for jb in range(2):
                    owj = owpool.tile([128, KC, OJ], BF16, tag="owj")
                    nc.sync.dma_start(
                        owj[:], owFr[:, :, jb * OJ:(jb + 1) * OJ])
                    tiles.append(owj)
                prefetched[b] = tiles

            def emit_proj(b):
                drain_att()
                for jb in range(C // OJ):
                    if jb < 2:
                        owj = prefetched[b][jb]
                    else:
                        owj = owpool.tile([128, KC, OJ], BF16, tag="owj")
                        nc.sync.dma_start(
                            owj[:], owFr[:, :, jb * OJ:(jb + 1) * OJ])
                    for h in range(HPC):
                        av = atn[(b, h)][:].rearrange("p (a u) -> p a u", u=16)
                        psy = flowps.tile([128, OJ], F32, tag="flow")
                        for u in range(KC):
                            nc.tensor.matmul(
                                psy[:], av[:, :, u], owj[:, u, :],
                                start=(u == 0), stop=(u == KC - 1))
                        ys = ypool.tile([128, OJ], F32, tag="ys")
                        nc.scalar.copy(ys[:], psy[:])
                        nc.sync.dma_start(
                            y[b * HPC + h, :, jb * OJ:(jb + 1) * OJ], ys[:])

            # software pipeline: qkv(step) overlaps attention(step-1);
            # out-proj(b) emitted right after attention (b, NTB-1)
            # software pipeline, attention(s-1) EMITTED BEFORE qkv(s): every
            # engine's in-order queue then serves the attention work (which
            # the PE needs now) before the next block's rope/cast work
            steps = [(s // NTB, s % NTB) for s in range(B * NTB)]
            for s in range(B * NTB + 1):
                if s >= 1:
                    bq, tbq = steps[s - 1]
                    if tbq == NTB - 1:
                        prefetch_proj(bq)
                    emit_attention(bq, tbq)
                    if tbq == NTB - 1:
                        emit_proj(bq)
                if s < B * NTB:
                    emit_qkv(*steps[s])
    nc.compile()
    return nc


def _get_nc():
    global _CACHED_NC
    if _CACHED_NC is None:
        _CACHED_NC = build_nc()
    return _CACHED_NC


def _rope_tables():
    pos = np.arange(T, dtype=np.float64)[:, None]
    div = np.exp(np.arange(0, D, 2, dtype=np.float64) *
                 (-math.log(10000.0) / D))
    ang = pos * div  # [T, 64]
    sinT = np.sin(ang).T.astype(np.float32)  # [64, T]
    cosT = np.cos(ang).T.astype(np.float32)
    cosF = np.ascontiguousarray(np.concatenate([cosT, cosT], axis=0))
    sinS = np.ascontiguousarray(np.concatenate([-sinT, sinT], axis=0))
    return cosF, sinS


def make_in_maps(x, qkv_w, out_w):
    xT = np.ascontiguousarray(x.reshape(BT, C).T.astype(NPBF))
    owF = np.ascontiguousarray(out_w.T.astype(NPBF))
    cosF, sinS = _rope_tables()
    ones = np.ones((128, 128), dtype=NPBF)
    tri = np.triu(np.ones((128, 128))).astype(NPBF)  # keep col >= key row
    in_maps = []
    for c in range(N_CORES):
        h0, h1 = 2 * c, 2 * c + 1
        wqk = np.concatenate([
            qkv_w[h0 * D:(h0 + 1) * D],
            qkv_w[h1 * D:(h1 + 1) * D],
            qkv_w[C + h0 * D:C + (h0 + 1) * D],
            qkv_w[C + h1 * D:C + (h1 + 1) * D],
        ], axis=0)                       # [512, 2048]
        wv = np.concatenate([
            qkv_w[2 * C + h0 * D:2 * C + (h0 + 1) * D],
            qkv_w[2 * C + h1 * D:2 * C + (h1 + 1) * D],
        ], axis=0)                       # [256, 2048]
        in_maps.append({
            "xT": xT,
            "wqkT": np.ascontiguousarray(wqk.T.astype(NPBF)),
            "wvT": np.ascontiguousarray(wv.T.astype(NPBF)),
            "owF": owF,
            "cosF": cosF,
            "sinS": sinS,
            "onesI": ones,
            "triM": tri,
        })
    return in_maps


def kernel(x, qkv_w, out_w, _trace=False, _trace_kwargs=None):
    x = np.asarray(x, dtype=np.float32)
    qkv_w = np.asarray(qkv_w, dtype=np.float32)
    out_w = np.asarray(out_w, dtype=np.float32)
    nc = _get_nc()
    in_maps = make_in_maps(x, qkv_w, out_w)
    kwargs = {}
    if _trace:
        kwargs["trace"] = True
        if _trace_kwargs:
            kwargs.update(_trace_kwargs)
    res = run_bass_kernel_spmd(nc, in_maps, core_ids=list(range(N_CORES)),
                               **kwargs)
    out = np.empty((B, T, C), dtype=np.float32)
    for c in range(N_CORES):
        yc = res.results[c]["y"]  # [B*HPC, 128, C]
        for b in range(B):
            for hl in range(HPC):
                hg = HPC * c + hl
                out[b, hg * 128:(hg + 1) * 128] = yc[b * HPC + hl]
    if _trace:
        return out, res
    return out
